# revision 17
# baseline (speedup 1.0000x reference)
# kernel.py — nn_CustomLinearEval: group-dequantized linear layer on 8 trn2 cores.
#
# out[b,s,n] = sum_k x[b,s,k] * w_dq[k,n] + bias[n]
#   w_dq = round(weight.T / s) * s,  s = step_scales[g,n] + 1e-8, g = k // 128
#
# Sharding: column-parallel (tensor-parallel over N). Each core owns 512 of the
# 4096 output features:
#   - The host pre-packs x into per-block SBUF images (x_img[mb*128+p,
#     kt*512+j] = x[mb*512+j, kt*128+p], bf16), so ONE 4 MiB DMA loads a whole
#     512-column m-block. DMA issue instructions cost ~0.7us of engine time
#     each, so going from 512 chunk-DMAs to 16 block-DMAs is what lets the
#     sync engine keep the x stream ahead of the PE.
#   - Each core DMAs its [512, 4096] fp32 weight shard in 8 chunks split
#     across the two HWDGE queues (sync/scalar) AHEAD of the x blocks, then
#     dequantizes in natural [n, k] layout with round-half-even via the
#     +/-1.5*2^23 magic trick (matching jnp.round): the scalar engine does
#     t1 = w*(1/s) + MAGIC (its native in*scale+bias form), DVE does
#     w_dq = (t1 - MAGIC)*s (subtract FIRST: s*t1 - MAGIC*s catastrophically
#     cancels). The 128 [n,k] tiles are transposed on the PE once; w_dq^T
#     (bf16, 4 MiB) stays SBUF-resident.
#   - Matmul phase: back-to-back bf16 matmuls (free dim 512, one PSUM bank
#     per 32-deep accumulation group). Block-0/1 passes are interleaved with
#     the phase-0 dequant rows in PE program order so the PE never idles
#     during warmup.
#   - Bias-add fuses into the PSUM->SBUF copy on the scalar engine; out DMAs
#     issue from the scalar engine's DGE (keeps the sync queue for x).
# Host gathers the 8 out^T row-shards and transposes once in numpy.

import numpy as np
import ml_dtypes

BF16 = ml_dtypes.bfloat16

GS = 128
EPS = 1e-8
B, S, K, N = 4, 2048, 4096, 4096
M = B * S
NCORES = 8
NL = N // NCORES          # 512 out-features per core
G = K // GS               # 32 quant groups
NT = NL // 128            # 4 n tiles per core
KT = K // 128             # 32 k tiles
MBS = [256, 256] + [512] * 14 + [256, 128, 128]   # m-block widths (sum = M)
MOFF = [sum(MBS[:i]) for i in range(len(MBS))]
NMB = len(MBS)
XROWW = 32 * 512          # x_img row width (max block image width)
MAGIC = float(np.float32(12582912.0))  # 1.5 * 2**23: fp32 round-to-nearest-even

_NC_CACHE = {}


def _build_nc():
    import concourse.bass as bass
    import concourse.mybir as mybir
    import concourse.tile as tile

    f32 = mybir.dt.float32
    bf16 = mybir.dt.bfloat16
    AF = mybir.ActivationFunctionType
    OP = mybir.AluOpType

    nc = bass.Bass()
    # x_img: host-pre-packed per-block SBUF images, [NMB*128, KT*512] bf16
    x_img = nc.dram_tensor("x_img", [NMB * 128, XROWW], bf16, kind="ExternalInput")
    w = nc.dram_tensor("w", [NL, K], f32, kind="ExternalInput")
    srep = nc.dram_tensor("srep", [128, NT * G], f32, kind="ExternalInput")
    rrep = nc.dram_tensor("rrep", [128, NT * G], f32, kind="ExternalInput")
    brep = nc.dram_tensor("brep", [128, NT], f32, kind="ExternalInput")
    ident = nc.dram_tensor("ident", [128, 128], bf16, kind="ExternalInput")
    out_t = nc.dram_tensor("out_t", [NL, M], f32, kind="ExternalOutput")

    WCH = 2048                # k-columns per weight DMA chunk (16 k-tiles)
    NGRP = KT // 4            # 8 transpose groups of 4 k-tiles per n row
    XW = KT * 512             # x block tile width (max)

    with tile.TileContext(nc) as tc:
        with (
            tc.tile_pool(name="const", bufs=1) as constp,
            tc.tile_pool(name="wdqT", bufs=1) as wdqTp,
            tc.tile_pool(name="xblk", bufs=3) as xp,
            tc.tile_pool(name="wnat", bufs=1) as wnatp,
            tc.tile_pool(name="t1", bufs=4) as t1p,
            tc.tile_pool(name="wdq", bufs=8) as wdqp,
            tc.tile_pool(name="outsb", bufs=3) as outp,
            tc.tile_pool(name="tp_ps", bufs=2, space="PSUM") as tpps,
            tc.tile_pool(name="acc_ps", bufs=3, space="PSUM") as accps,
        ):
            # consts first (tiny, on the fast HWDGE queues)
            r_sb = constp.tile([128, NT * G], f32)
            nc.sync.dma_start(r_sb[:], rrep[:, :])
            s_sb = constp.tile([128, NT * G], f32)
            nc.scalar.dma_start(s_sb[:], srep[:, :])
            id_sb = constp.tile([128, 128], bf16)
            nc.sync.dma_start(id_sb[:], ident[:, :])
            b_sb = constp.tile([128, NT], f32)
            nc.scalar.dma_start(b_sb[:], brep[:, :])
            magic_sb = constp.tile([128, 1], f32)
            nc.gpsimd.memset(magic_sb[:], MAGIC)

            # persistent dequantized-transposed weight tiles: [k=128, n 4*128]
            # per group of 4 k-tiles; wdqT[nt*NGRP + kt//4][:, (kt%4)*128...]
            wdqT = [
                wdqTp.tile([128, 512], bf16, name=f"wdqT{i}")
                for i in range(NT * NGRP)
            ]

            # weight chunks: chunk c0 of each row on sync, c1 on scalar.
            # Emission interleaves the first x blocks between weight rows so
            # the 12 MB startup payload keeps both queues busy in the order
            # the PE will consume it: w r0, xb0, xb1 (small ramp blocks),
            # w r1.., xb2...
            wns = {}

            def w_issue(nt, nch=2):
                cw = K // nch
                tiles = []
                for c in range(nch):
                    wn = wnatp.tile(
                        [128, cw], f32, tag=f"wn{nt}_{c}", name=f"wn{nt}_{c}"
                    )
                    eng = nc.sync if c % 2 == 0 else nc.scalar
                    eng.dma_start(
                        wn[:], w[nt * 128 : (nt + 1) * 128, c * cw : (c + 1) * cw]
                    )
                    tiles.append(wn)
                wns[nt] = (cw, tiles)

            def wslice(nt, kt):
                cw, tiles = wns[nt]
                c, off = (kt * 128) // cw, (kt * 128) % cw
                return tiles[c][:, off : off + 128]

            xbs = {}

            def x_issue(mb, split=False):
                mw = MBS[mb]
                xbs[mb] = xp.tile([128, XW], bf16, tag="xblk", name=f"xb{mb}")
                if split:
                    h = KT * mw // 2
                    nc.sync.dma_start(
                        xbs[mb][:, :h], x_img[mb * 128 : (mb + 1) * 128, :h]
                    )
                    nc.scalar.dma_start(
                        xbs[mb][:, h : 2 * h],
                        x_img[mb * 128 : (mb + 1) * 128, h : 2 * h],
                    )
                else:
                    nc.sync.dma_start(
                        xbs[mb][:, : KT * mw],
                        x_img[mb * 128 : (mb + 1) * 128, : KT * mw],
                    )

            w_issue(0, nch=4)
            x_issue(0, split=True)
            x_issue(1, split=True)
            w_issue(1)
            x_issue(2, split=True)
            w_issue(2)
            w_issue(3)
            x_issue(3)

            def mm_pass(mb, nt):
                xb = xbs[mb]
                m0, mw = MOFF[mb], MBS[mb]
                acc = accps.tile([128, mw], f32, tag="acc")
                for kt in range(KT):
                    grp = nt * NGRP + kt // 4
                    sub = kt % 4
                    nc.tensor.matmul(
                        acc[:],
                        wdqT[grp][:, sub * 128 : (sub + 1) * 128],
                        xb[:, kt * mw : (kt + 1) * mw],
                        start=(kt == 0),
                        stop=(kt == KT - 1),
                    )
                outsb = outp.tile([128, mw], f32, tag="outsb")
                nc.scalar.activation(
                    outsb[:], acc[:], AF.Identity,
                    bias=b_sb[:, nt : nt + 1], scale=1.0,
                )
                nc.scalar.dma_start(
                    out_t[nt * 128 : (nt + 1) * 128, m0 : m0 + mw], outsb[:]
                )

            P0_SCHED = {
                1: ((0, 0), (0, 1)),
                2: ((1, 0), (1, 1)),
                3: ((0, 2), (1, 2), (0, 3), (1, 3)),
            }

            # ---- phase 0 (dequant+transpose), interleaved with mb0-mb3 MMs
            for nt in range(NT):
                for kt in range(KT):
                        col = nt * G + kt
                        grp = nt * NGRP + kt // 4
                        sub = kt % 4
                        wsl = wslice(nt, kt)
                        # t1 = (w * (1/s)) + MAGIC; mostly on the scalar
                        # engine (native in*scale+bias form), every 4th on
                        # DVE to balance the two engines' row latency
                        t1 = t1p.tile([128, 128], f32)
                        if kt % 4 == 3:
                            nc.vector.tensor_scalar(
                                t1[:],
                                wsl,
                                r_sb[:, col : col + 1],
                                MAGIC,
                                op0=OP.mult,
                                op1=OP.add,
                            )
                        else:
                            nc.scalar.activation(
                                t1[:],
                                wsl,
                                AF.Identity,
                                bias=magic_sb[:],
                                scale=r_sb[:, col : col + 1],
                            )
                        # w_dq = (t1 - MAGIC) * s on DVE (subtract FIRST:
                        # s*t1 - MAGIC*s catastrophically cancels)
                        wdq = wdqp.tile([128, 128], bf16)
                        nc.vector.tensor_scalar(
                            wdq[:],
                            t1[:],
                            MAGIC,
                            s_sb[:, col : col + 1],
                            op0=OP.subtract,
                            op1=OP.mult,
                        )
                        if sub == 0:
                            ps = tpps.tile([128, 512], bf16)
                        nc.tensor.transpose(
                            ps[:, sub * 128 : (sub + 1) * 128], wdq[:], id_sb[:]
                        )
                        if sub == 3:
                            nc.scalar.copy(wdqT[grp][:], ps[:])
                # ramp-block passes are emitted per the interleave schedule
                for bmb, bnt in P0_SCHED.get(nt, ()):
                    mm_pass(bmb, bnt)

            # ---- phase 1: stream the remaining m blocks ----
            for mb in range(2, NMB):
                nxt = mb + 2
                if nxt < NMB:
                    x_issue(nxt)
                for nt in range(NT):
                    mm_pass(mb, nt)

    _split_waits(nc)
    return nc


def _split_waits(nc, max_waits=1):
    """The walrus build in this container rejects >1 sync-wait per instruction
    ("Too many sync wait commands"). Hoist extra waits onto preceding
    same-engine NOPs, which is semantically identical (in-order engines)."""
    import concourse.mybir as mybir

    for func in nc.m.functions:
        for bb in func.blocks:
            insts = list(bb.instructions)
            new_insts = []
            changed = False
            for inst in insts:
                si = inst.sync_info
                waits = list(si.on_wait) if si is not None and si.on_wait else []
                if len(waits) > max_waits:
                    keep = waits[-max_waits:]
                    for j, wcond in enumerate(waits[:-max_waits]):
                        new_insts.append(
                            mybir.InstNoOp(
                                name=f"{inst.name}-ws{j}",
                                engine=inst.engine,
                                sync_info=mybir.SyncInfo(on_wait=[wcond], on_update=[]),
                            )
                        )
                    si.on_wait = keep
                    inst.sync_info = si
                    changed = True
                new_insts.append(inst)
            if changed:
                bb.instructions = new_insts


def _prep_inputs(x, weight, bias, step_scales):
    x = np.ascontiguousarray(np.asarray(x, dtype=np.float32)).reshape(M, K)
    weight = np.ascontiguousarray(np.asarray(weight, dtype=np.float32))
    bias = np.asarray(bias, dtype=np.float32)
    step_scales = np.asarray(step_scales, dtype=np.float32)

    s_eff = (step_scales + np.float32(EPS)).astype(np.float32)      # [G, N]
    recip = (np.float32(1.0) / s_eff).astype(np.float32)            # [G, N]

    # per-block SBUF images: x_img[mb*128+p, kt*mw+j] = x[m0+j, kt*128+p]
    x_img = np.zeros((NMB * 128, XROWW), dtype=BF16)
    for mb in range(NMB):
        m0, mw = MOFF[mb], MBS[mb]
        blk = (
            x[m0 : m0 + mw, :]
            .reshape(mw, KT, 128)
            .transpose(2, 1, 0)
            .reshape(128, KT * mw)
        )
        x_img[mb * 128 : (mb + 1) * 128, : KT * mw] = blk.astype(BF16)
    ident = np.eye(128, dtype=BF16)

    def rep(a):  # [G, NL] -> [128, NT*G] with col nt*G+g = a[g, nt*128+p]
        return np.ascontiguousarray(
            a.T.reshape(NT, 128, G).transpose(1, 0, 2).reshape(128, NT * G)
        )

    in_maps = []
    for c in range(NCORES):
        n0 = c * NL
        sl = slice(n0, n0 + NL)
        in_maps.append(
            {
                "x_img": x_img,
                "w": np.ascontiguousarray(weight[sl, :]),
                "srep": rep(s_eff[:, sl]),
                "rrep": rep(recip[:, sl]),
                "brep": np.ascontiguousarray(bias[sl].reshape(NT, 128).T),
                "ident": ident,
            }
        )
    return in_maps


def run_on_hw(x, weight, bias, step_scales, trace=False, **kw):
    from concourse.bass_utils import run_bass_kernel_spmd

    if "nc" not in _NC_CACHE:
        _NC_CACHE["nc"] = _build_nc()
    nc = _NC_CACHE["nc"]
    in_maps = _prep_inputs(x, weight, bias, step_scales)
    res = run_bass_kernel_spmd(
        nc, in_maps, core_ids=list(range(NCORES)), trace=trace, **kw
    )
    out_t = np.concatenate([res.results[c]["out_t"] for c in range(NCORES)], axis=0)
    out = np.ascontiguousarray(out_t.T).reshape(B, S, N)
    return out, res


def kernel(x, weight, bias, step_scales):
    out, _ = run_on_hw(x, weight, bias, step_scales, trace=False)
    return out

assert sum(MBS) == M


# revision 18
# speedup vs baseline: 1.0101x; 1.0101x over previous
# kernel.py — nn_CustomLinearEval: group-dequantized linear layer on 8 trn2 cores.
#
# out[b,s,n] = sum_k x[b,s,k] * w_dq[k,n] + bias[n]
#   w_dq = round(weight.T / s) * s,  s = step_scales[g,n] + 1e-8, g = k // 128
#
# Sharding: column-parallel (tensor-parallel over N). Each core owns 512 of the
# 4096 output features:
#   - The host pre-packs x into per-block SBUF images (x_img[mb*128+p,
#     kt*512+j] = x[mb*512+j, kt*128+p], bf16), so ONE 4 MiB DMA loads a whole
#     512-column m-block. DMA issue instructions cost ~0.7us of engine time
#     each, so going from 512 chunk-DMAs to 16 block-DMAs is what lets the
#     sync engine keep the x stream ahead of the PE.
#   - Each core DMAs its [512, 4096] fp32 weight shard in 8 chunks split
#     across the two HWDGE queues (sync/scalar) AHEAD of the x blocks, then
#     dequantizes in natural [n, k] layout with round-half-even via the
#     +/-1.5*2^23 magic trick (matching jnp.round): the scalar engine does
#     t1 = w*(1/s) + MAGIC (its native in*scale+bias form), DVE does
#     w_dq = (t1 - MAGIC)*s (subtract FIRST: s*t1 - MAGIC*s catastrophically
#     cancels). The 128 [n,k] tiles are transposed on the PE once; w_dq^T
#     (bf16, 4 MiB) stays SBUF-resident.
#   - Matmul phase: back-to-back bf16 matmuls (free dim 512, one PSUM bank
#     per 32-deep accumulation group). Block-0/1 passes are interleaved with
#     the phase-0 dequant rows in PE program order so the PE never idles
#     during warmup.
#   - Bias-add fuses into the PSUM->SBUF copy on the scalar engine; out DMAs
#     issue from the scalar engine's DGE (keeps the sync queue for x).
# Host gathers the 8 out^T row-shards and transposes once in numpy.

import numpy as np
import ml_dtypes

BF16 = ml_dtypes.bfloat16

GS = 128
EPS = 1e-8
B, S, K, N = 4, 2048, 4096, 4096
M = B * S
NCORES = 8
NL = N // NCORES          # 512 out-features per core
G = K // GS               # 32 quant groups
NT = NL // 128            # 4 n tiles per core
KT = K // 128             # 32 k tiles
MBS = [256, 256] + [512] * 14 + [256, 128, 128]   # m-block widths (sum = M)
MOFF = [sum(MBS[:i]) for i in range(len(MBS))]
NMB = len(MBS)
XROWW = 32 * 512          # x_img row width (max block image width)
MAGIC = float(np.float32(12582912.0))  # 1.5 * 2**23: fp32 round-to-nearest-even

_NC_CACHE = {}


def _build_nc():
    import concourse.bass as bass
    import concourse.mybir as mybir
    import concourse.tile as tile

    f32 = mybir.dt.float32
    bf16 = mybir.dt.bfloat16
    AF = mybir.ActivationFunctionType
    OP = mybir.AluOpType

    nc = bass.Bass()
    # x_img: host-pre-packed per-block SBUF images, [NMB*128, KT*512] bf16
    x_img = nc.dram_tensor("x_img", [NMB * 128, XROWW], bf16, kind="ExternalInput")
    w = nc.dram_tensor("w", [NL, K], f32, kind="ExternalInput")
    srep = nc.dram_tensor("srep", [128, NT * G], f32, kind="ExternalInput")
    rrep = nc.dram_tensor("rrep", [128, NT * G], f32, kind="ExternalInput")
    brep = nc.dram_tensor("brep", [128, NT], f32, kind="ExternalInput")
    ident = nc.dram_tensor("ident", [128, 128], bf16, kind="ExternalInput")
    out_t = nc.dram_tensor("out_t", [NL, M], f32, kind="ExternalOutput")

    WCH = 2048                # k-columns per weight DMA chunk (16 k-tiles)
    NGRP = KT // 4            # 8 transpose groups of 4 k-tiles per n row
    XW = KT * 512             # x block tile width (max)

    with tile.TileContext(nc) as tc:
        with (
            tc.tile_pool(name="const", bufs=1) as constp,
            tc.tile_pool(name="wdqT", bufs=1) as wdqTp,
            tc.tile_pool(name="xblk", bufs=3) as xp,
            tc.tile_pool(name="wnat", bufs=1) as wnatp,
            tc.tile_pool(name="t1", bufs=4) as t1p,
            tc.tile_pool(name="wdq", bufs=8) as wdqp,
            tc.tile_pool(name="outsb", bufs=3) as outp,
            tc.tile_pool(name="tp_ps", bufs=2, space="PSUM") as tpps,
            tc.tile_pool(name="acc_ps", bufs=3, space="PSUM") as accps,
        ):
            # consts first (tiny, on the fast HWDGE queues)
            r_sb = constp.tile([128, NT * G], f32)
            nc.sync.dma_start(r_sb[:], rrep[:, :])
            s_sb = constp.tile([128, NT * G], f32)
            nc.scalar.dma_start(s_sb[:], srep[:, :])
            id_sb = constp.tile([128, 128], bf16)
            nc.sync.dma_start(id_sb[:], ident[:, :])
            b_sb = constp.tile([128, NT], f32)
            nc.scalar.dma_start(b_sb[:], brep[:, :])
            magic_sb = constp.tile([128, 1], f32)
            nc.gpsimd.memset(magic_sb[:], MAGIC)

            # persistent dequantized-transposed weight tiles: [k=128, n 4*128]
            # per group of 4 k-tiles; wdqT[nt*NGRP + kt//4][:, (kt%4)*128...]
            wdqT = [
                wdqTp.tile([128, 512], bf16, name=f"wdqT{i}")
                for i in range(NT * NGRP)
            ]

            # weight chunks: chunk c0 of each row on sync, c1 on scalar.
            # Emission interleaves the first x blocks between weight rows so
            # the 12 MB startup payload keeps both queues busy in the order
            # the PE will consume it: w r0, xb0, xb1 (small ramp blocks),
            # w r1.., xb2...
            wns = {}

            def w_issue(nt, nch=2):
                cw = K // nch
                tiles = []
                for c in range(nch):
                    wn = wnatp.tile(
                        [128, cw], f32, tag=f"wn{nt}_{c}", name=f"wn{nt}_{c}"
                    )
                    eng = nc.sync if c % 2 == 0 else nc.scalar
                    eng.dma_start(
                        wn[:], w[nt * 128 : (nt + 1) * 128, c * cw : (c + 1) * cw]
                    )
                    tiles.append(wn)
                wns[nt] = (cw, tiles)

            def wslice(nt, kt):
                cw, tiles = wns[nt]
                c, off = (kt * 128) // cw, (kt * 128) % cw
                return tiles[c][:, off : off + 128]

            xbs = {}

            def x_issue(mb, split=False):
                mw = MBS[mb]
                xbs[mb] = xp.tile([128, XW], bf16, tag="xblk", name=f"xb{mb}")
                if split:
                    h = KT * mw // 2
                    nc.sync.dma_start(
                        xbs[mb][:, :h], x_img[mb * 128 : (mb + 1) * 128, :h]
                    )
                    nc.scalar.dma_start(
                        xbs[mb][:, h : 2 * h],
                        x_img[mb * 128 : (mb + 1) * 128, h : 2 * h],
                    )
                else:
                    nc.sync.dma_start(
                        xbs[mb][:, : KT * mw],
                        x_img[mb * 128 : (mb + 1) * 128, : KT * mw],
                    )

            w_issue(0, nch=4)
            x_issue(0)
            x_issue(1)
            w_issue(1)
            x_issue(2)
            w_issue(2)
            w_issue(3)
            x_issue(3)

            def mm_pass(mb, nt):
                xb = xbs[mb]
                m0, mw = MOFF[mb], MBS[mb]
                acc = accps.tile([128, mw], f32, tag="acc")
                for kt in range(KT):
                    grp = nt * NGRP + kt // 4
                    sub = kt % 4
                    nc.tensor.matmul(
                        acc[:],
                        wdqT[grp][:, sub * 128 : (sub + 1) * 128],
                        xb[:, kt * mw : (kt + 1) * mw],
                        start=(kt == 0),
                        stop=(kt == KT - 1),
                    )
                outsb = outp.tile([128, mw], f32, tag="outsb")
                nc.scalar.activation(
                    outsb[:], acc[:], AF.Identity,
                    bias=b_sb[:, nt : nt + 1], scale=1.0,
                )
                nc.scalar.dma_start(
                    out_t[nt * 128 : (nt + 1) * 128, m0 : m0 + mw], outsb[:]
                )

            P0_SCHED = {
                1: ((0, 0), (0, 1)),
                2: ((1, 0), (1, 1)),
                3: ((0, 2), (1, 2), (0, 3), (1, 3)),
            }

            # ---- phase 0 (dequant+transpose), interleaved with mb0-mb3 MMs
            for nt in range(NT):
                for kt in range(KT):
                        col = nt * G + kt
                        grp = nt * NGRP + kt // 4
                        sub = kt % 4
                        wsl = wslice(nt, kt)
                        # t1 = (w * (1/s)) + MAGIC; mostly on the scalar
                        # engine (native in*scale+bias form), every 4th on
                        # DVE to balance the two engines' row latency
                        t1 = t1p.tile([128, 128], f32)
                        if kt % 4 == 3:
                            nc.vector.tensor_scalar(
                                t1[:],
                                wsl,
                                r_sb[:, col : col + 1],
                                MAGIC,
                                op0=OP.mult,
                                op1=OP.add,
                            )
                        else:
                            nc.scalar.activation(
                                t1[:],
                                wsl,
                                AF.Identity,
                                bias=magic_sb[:],
                                scale=r_sb[:, col : col + 1],
                            )
                        # w_dq = (t1 - MAGIC) * s on DVE (subtract FIRST:
                        # s*t1 - MAGIC*s catastrophically cancels)
                        wdq = wdqp.tile([128, 128], bf16)
                        nc.vector.tensor_scalar(
                            wdq[:],
                            t1[:],
                            MAGIC,
                            s_sb[:, col : col + 1],
                            op0=OP.subtract,
                            op1=OP.mult,
                        )
                        if sub == 0:
                            ps = tpps.tile([128, 512], bf16)
                        nc.tensor.transpose(
                            ps[:, sub * 128 : (sub + 1) * 128], wdq[:], id_sb[:]
                        )
                        if sub == 3:
                            nc.scalar.copy(wdqT[grp][:], ps[:])
                # ramp-block passes are emitted per the interleave schedule
                for bmb, bnt in P0_SCHED.get(nt, ()):
                    mm_pass(bmb, bnt)

            # ---- phase 1: stream the remaining m blocks ----
            for mb in range(2, NMB):
                nxt = mb + 2
                if nxt < NMB:
                    x_issue(nxt)
                for nt in range(NT):
                    mm_pass(mb, nt)

    _split_waits(nc)
    return nc


def _split_waits(nc, max_waits=1):
    """The walrus build in this container rejects >1 sync-wait per instruction
    ("Too many sync wait commands"). Hoist extra waits onto preceding
    same-engine NOPs, which is semantically identical (in-order engines)."""
    import concourse.mybir as mybir

    for func in nc.m.functions:
        for bb in func.blocks:
            insts = list(bb.instructions)
            new_insts = []
            changed = False
            for inst in insts:
                si = inst.sync_info
                waits = list(si.on_wait) if si is not None and si.on_wait else []
                if len(waits) > max_waits:
                    keep = waits[-max_waits:]
                    for j, wcond in enumerate(waits[:-max_waits]):
                        new_insts.append(
                            mybir.InstNoOp(
                                name=f"{inst.name}-ws{j}",
                                engine=inst.engine,
                                sync_info=mybir.SyncInfo(on_wait=[wcond], on_update=[]),
                            )
                        )
                    si.on_wait = keep
                    inst.sync_info = si
                    changed = True
                new_insts.append(inst)
            if changed:
                bb.instructions = new_insts


def _prep_inputs(x, weight, bias, step_scales):
    x = np.ascontiguousarray(np.asarray(x, dtype=np.float32)).reshape(M, K)
    weight = np.ascontiguousarray(np.asarray(weight, dtype=np.float32))
    bias = np.asarray(bias, dtype=np.float32)
    step_scales = np.asarray(step_scales, dtype=np.float32)

    s_eff = (step_scales + np.float32(EPS)).astype(np.float32)      # [G, N]
    recip = (np.float32(1.0) / s_eff).astype(np.float32)            # [G, N]

    # per-block SBUF images: x_img[mb*128+p, kt*mw+j] = x[m0+j, kt*128+p]
    x_img = np.zeros((NMB * 128, XROWW), dtype=BF16)
    for mb in range(NMB):
        m0, mw = MOFF[mb], MBS[mb]
        blk = (
            x[m0 : m0 + mw, :]
            .reshape(mw, KT, 128)
            .transpose(2, 1, 0)
            .reshape(128, KT * mw)
        )
        x_img[mb * 128 : (mb + 1) * 128, : KT * mw] = blk.astype(BF16)
    ident = np.eye(128, dtype=BF16)

    def rep(a):  # [G, NL] -> [128, NT*G] with col nt*G+g = a[g, nt*128+p]
        return np.ascontiguousarray(
            a.T.reshape(NT, 128, G).transpose(1, 0, 2).reshape(128, NT * G)
        )

    in_maps = []
    for c in range(NCORES):
        n0 = c * NL
        sl = slice(n0, n0 + NL)
        in_maps.append(
            {
                "x_img": x_img,
                "w": np.ascontiguousarray(weight[sl, :]),
                "srep": rep(s_eff[:, sl]),
                "rrep": rep(recip[:, sl]),
                "brep": np.ascontiguousarray(bias[sl].reshape(NT, 128).T),
                "ident": ident,
            }
        )
    return in_maps


def run_on_hw(x, weight, bias, step_scales, trace=False, **kw):
    from concourse.bass_utils import run_bass_kernel_spmd

    if "nc" not in _NC_CACHE:
        _NC_CACHE["nc"] = _build_nc()
    nc = _NC_CACHE["nc"]
    in_maps = _prep_inputs(x, weight, bias, step_scales)
    res = run_bass_kernel_spmd(
        nc, in_maps, core_ids=list(range(NCORES)), trace=trace, **kw
    )
    out_t = np.concatenate([res.results[c]["out_t"] for c in range(NCORES)], axis=0)
    out = np.ascontiguousarray(out_t.T).reshape(B, S, N)
    return out, res


def kernel(x, weight, bias, step_scales):
    out, _ = run_on_hw(x, weight, bias, step_scales, trace=False)
    return out

assert sum(MBS) == M
